# revision 6
# baseline (speedup 1.0000x reference)
"""HSMM forward-pass kernel for Trainium2 (8 NeuronCores, data-parallel over batch).

Algorithm: the explicit-duration HSMM forward recurrence is rewritten in
exp-space, where it becomes a *linear* recurrence with time-varying diagonal
scalings:

    w_t = EC2_t * ( expA^T @ (EC1_t * sum_{d=1..64} expD_d * w_{t-d}) )

with all EC* factors precomputable on the host from cumsum(log_B).  Numerical
range is controlled by per-j anchors Phi_n(j) = cumb at 64-step block
boundaries (host precomputed) plus one runtime scalar per block per sequence
(mu_n), estimated on-device with a partition-sum probe and applied as a ring
rescale at each block boundary.  alpha_t = log(G_t) + cumb_t - Phi_n + mu_n
is reconstructed afterwards (log on device, the rest on host).

Each core handles 2 sequences; K=256 states are split into 2 partition-halves
(g = 2*h + b indexes the 4 (half, seq) groups laid out along the free dim).
The 64-tap duration filter runs on the Vector engine against a linear (non
circular) ring buffer that is repacked/rescaled once per 64-step block; the
transition matvec runs on the Tensor engine with bf16 weights.
"""

import os
import sys

import numpy as np

sys.path.insert(0, "/opt/trn_rl_repo")

import ml_dtypes

BF = ml_dtypes.bfloat16

B, T, K, DMAX = 16, 2048, 256, 64
NCORES = 8
SPC = B // NCORES  # sequences per core = 2
BLK = 64  # block length (= DMAX)
PROBE_K = 32  # probe offset within a block


# ---------------------------------------------------------------- host precompute
def _precompute(log_B, pi_logits, A_logits, D_logits, t_steps=T):
    """Build per-core device input arrays + host-side reconstruction arrays."""
    nblk = t_steps // BLK
    logb = np.asarray(log_B, dtype=np.float64)[:, :t_steps, :]
    cumb = np.cumsum(logb, axis=1)  # [B, t, K] float64

    # anchors Phi_n = cumb at block starts (cumb_{64n-1}; 0 for n=0)
    phi = np.zeros((B, nblk, K), dtype=np.float64)
    for n in range(1, nblk):
        phi[:, n, :] = cumb[:, BLK * n - 1, :]

    nidx = np.arange(t_steps) // BLK
    phit = phi[:, nidx, :]  # [B, t, K]
    ec1 = np.exp(cumb - phit)  # [B, t, K]
    ec2 = np.exp(phit - cumb)
    fr = np.ones((B, nblk, K), dtype=np.float64)
    if nblk > 1:
        fr[:, : nblk - 1, :] = np.exp(phi[:, 1:, :] - phi[:, : nblk - 1, :])

    exppi = np.exp(np.asarray(pi_logits, dtype=np.float64))  # [K]
    expA = np.exp(np.asarray(A_logits, dtype=np.float64))  # [i, j]
    expD = np.exp(np.asarray(D_logits, dtype=np.float64))  # [j, d-1]

    # device layouts ------------------------------------------------------
    def to_dev(x_btj, dtype):
        # [b, t, K] -> [128p, g=(h,b), t]
        b_, t_, _ = x_btj.shape
        v = x_btj.reshape(b_, t_, 2, 128).transpose(3, 2, 0, 1)  # [p, h, b, t]
        return np.ascontiguousarray(v.reshape(128, 2 * b_, t_)).astype(dtype)

    edrev = np.empty((128, 4, DMAX), dtype=np.float64)
    for h in range(2):
        for b in range(2):
            edrev[:, 2 * h + b, :] = expD[h * 128 : (h + 1) * 128, ::-1]
    wexp = np.empty((128, 2, 256), dtype=np.float64)
    for hi in range(2):
        wexp[:, hi, :] = expA[hi * 128 : (hi + 1) * 128, :]

    ring0 = np.zeros((128, 4, 2 * BLK), dtype=np.float64)
    for h in range(2):
        for b in range(2):
            ring0[:, 2 * h + b, BLK - 1] = exppi[h * 128 : (h + 1) * 128]

    per_core = []
    for c in range(NCORES):
        bs = slice(SPC * c, SPC * (c + 1))
        per_core.append(
            dict(
                ec1=to_dev(ec1[bs], BF),
                ec1x=to_dev(ec1[bs] * (2.0**-40), np.float32),
                ec2=to_dev(ec2[bs], BF),
                fr=to_dev(fr[bs], BF),
                ring0=ring0.astype(BF),
                edrev=edrev.astype(BF),
                wexp=wexp.astype(BF),
                ones_k=np.ones((128, 1), dtype=BF),
                ones_m=np.ones((1, 128), dtype=np.float32),
            )
        )
    host = dict(nblk=nblk)
    return per_core, host


# ---------------------------------------------------------------- device kernel
def _build(t_steps=T, debug=False):
    import concourse.bass as bass
    import concourse.mybir as mybir
    from concourse import bacc, tile

    fp32 = mybir.dt.float32
    bf16 = mybir.dt.bfloat16
    MUL = mybir.AluOpType.mult
    nblk = t_steps // BLK

    nc = bacc.Bacc("TRN2", target_bir_lowering=False, debug=debug)

    ec1_d = nc.dram_tensor("ec1", [128, 4, t_steps], bf16, kind="ExternalInput")
    ec1x_d = nc.dram_tensor("ec1x", [128, 4, t_steps], fp32, kind="ExternalInput")
    ec2_d = nc.dram_tensor("ec2", [128, 4, t_steps], bf16, kind="ExternalInput")
    fr_d = nc.dram_tensor("fr", [128, 4, nblk], bf16, kind="ExternalInput")
    ring0_d = nc.dram_tensor("ring0", [128, 4, 2 * BLK], bf16, kind="ExternalInput")
    edrev_d = nc.dram_tensor("edrev", [128, 4, DMAX], bf16, kind="ExternalInput")
    wexp_d = nc.dram_tensor("wexp", [128, 2, 256], bf16, kind="ExternalInput")
    ones_k_d = nc.dram_tensor("ones_k", [128, 1], bf16, kind="ExternalInput")
    ones_m_d = nc.dram_tensor("ones_m", [1, 128], fp32, kind="ExternalInput")

    araw_d = nc.dram_tensor("alpha_raw", [128, 4, t_steps], fp32, kind="ExternalOutput")
    sprb_d = nc.dram_tensor("sprobe", [1, 2, nblk], fp32, kind="ExternalOutput")

    with tile.TileContext(nc) as tc:
        with (
            tc.tile_pool(name="persist", bufs=1) as pp,
            tc.tile_pool(name="work", bufs=3) as wp,
            tc.tile_pool(name="psum", bufs=2, space="PSUM") as pfp,
            tc.tile_pool(name="psum_s", bufs=2, space="PSUM") as psp,
        ):
            ec1 = pp.tile([128, 4, t_steps], bf16, tag="ec1")
            ec1x = pp.tile([128, 4, t_steps], fp32, tag="ec1x")
            ec2 = pp.tile([128, 4, t_steps], bf16, tag="ec2")
            fr = pp.tile([128, 4, nblk], bf16, tag="fr")
            ring = pp.tile([128, 4, 2 * BLK], bf16, tag="ring")
            edrev = pp.tile([128, 4, DMAX], bf16, tag="edrev")
            wexp = pp.tile([128, 2, 256], bf16, tag="wexp")
            ones_k = pp.tile([128, 1], bf16, tag="ones_k")
            ones_m = pp.tile([1, 128], fp32, tag="ones_m")
            gbuf = pp.tile([128, 4, t_steps], fp32, tag="gbuf")
            sout = pp.tile([1, 2, nblk], fp32, tag="sout")
            inv_s = pp.tile([1, 2], fp32, tag="inv_s")

            nc.sync.dma_start(ec1[:], ec1_d[:])
            nc.sync.dma_start(ec1x[:], ec1x_d[:])
            nc.sync.dma_start(ec2[:], ec2_d[:])
            nc.sync.dma_start(fr[:], fr_d[:])
            nc.sync.dma_start(ring[:], ring0_d[:])
            nc.sync.dma_start(edrev[:], edrev_d[:])
            nc.sync.dma_start(wexp[:], wexp_d[:])
            nc.sync.dma_start(ones_k[:], ones_k_d[:])
            nc.sync.dma_start(ones_m[:], ones_m_d[:])
            nc.vector.memset(sout[:], 0.0)

            for t in range(t_steps):
                n, k = divmod(t, BLK)
                # duration filter: products + reduce -> G[:, :, t]
                prod = wp.tile([128, 4, DMAX], bf16, tag="prod")
                nc.vector.tensor_mul(prod[:], ring[:, :, k : k + DMAX], edrev[:])
                nc.vector.tensor_reduce(
                    gbuf[:, :, t],
                    prod[:],
                    axis=mybir.AxisListType.X,
                    op=mybir.AluOpType.add,
                )
                # h column
                hcol = wp.tile([128, 4], bf16, tag="hcol")
                nc.vector.tensor_mul(hcol[:], gbuf[:, :, t], ec1[:, :, t])
                # transition matvec: f = expA^T @ h  (4 matmuls, psum accumulate)
                pf = pfp.tile([128, 4], fp32, tag="pf")
                for hj in range(2):
                    for hi in range(2):
                        nc.tensor.matmul(
                            pf[:, 2 * hj : 2 * hj + 2],
                            wexp[:, hi, hj * 128 : (hj + 1) * 128],
                            hcol[:, 2 * hi : 2 * hi + 2],
                            start=(hi == 0),
                            stop=(hi == 1),
                        )
                # w column -> ring
                nc.vector.tensor_mul(ring[:, :, BLK + k], pf[:], ec2[:, :, t])

                if k == PROBE_K and n < nblk - 1:
                    # probe: S[b] = sum_j h~ (both halves accumulate in psum)
                    ps = psp.tile([1, 2], fp32, tag="ps")
                    nc.tensor.matmul(
                        ps[:], ones_k[:], hcol[:, 0:2], start=True, stop=False
                    )
                    nc.tensor.matmul(
                        ps[:], ones_k[:], hcol[:, 2:4], start=False, stop=True
                    )
                    nc.vector.tensor_copy(sout[:, :, n], ps[0:1, :])
                    nc.vector.reciprocal(inv_s[:], ps[0:1, :])

                if k == BLK - 1 and n < nblk - 1:
                    # block boundary: rescale+repack ring, zero upper half
                    pb = psp.tile([128, 4], fp32, tag="pb")
                    rhs_b = inv_s[0:1, :].unsqueeze(1).broadcast_to([1, 2, 2])
                    nc.tensor.matmul(pb[:], ones_m[:], rhs_b, start=True, stop=True)
                    frmu = wp.tile([128, 4], bf16, tag="frmu")
                    nc.vector.tensor_mul(frmu[:], fr[:, :, n], pb[:])
                    nc.vector.tensor_mul(
                        ring[:, :, 0:BLK],
                        ring[:, :, BLK : 2 * BLK],
                        frmu[:].unsqueeze(2).broadcast_to([128, 4, BLK]),
                    )
                    nc.gpsimd.memset(ring[:, :, BLK : 2 * BLK], 0.0)

            # alpha-centered output: G * EC1 * 2^-40, then log, in place
            nc.vector.tensor_mul(gbuf[:], gbuf[:], ec1x[:])
            nc.scalar.activation(
                gbuf[:], gbuf[:], mybir.ActivationFunctionType.Ln
            )
            nc.sync.dma_start(araw_d[:], gbuf[:])
            nc.sync.dma_start(sprb_d[:], sout[:])

    nc.compile()
    return nc


# ---------------------------------------------------------------- golden numpy sim
def _golden(core_in, t_steps=T):
    """Replicates device ops (incl. bf16 rounding) for one core in numpy."""
    nblk = t_steps // BLK
    f32 = np.float32

    def bf(x):
        return x.astype(BF).astype(f32)

    ring = core_in["ring0"].astype(f32)  # [128, 4, 128]
    ec1x = core_in["ec1x"].astype(f32)
    edrev = core_in["edrev"].astype(f32)
    wexp = core_in["wexp"].astype(f32)
    ec1 = core_in["ec1"].astype(f32)
    ec2 = core_in["ec2"].astype(f32)
    fr = core_in["fr"].astype(f32)
    gbuf = np.zeros((128, 4, t_steps), f32)
    sout = np.zeros((1, 2, nblk), f32)

    for t in range(t_steps):
        n, k = divmod(t, BLK)
        prod = bf(ring[:, :, k : k + DMAX] * edrev)
        g = prod.sum(axis=2, dtype=f32)
        gbuf[:, :, t] = g
        hcol = bf(g * ec1[:, :, t])
        # f = expA^T h per (hj, b): contract over i = (hi, p)
        pf = np.zeros((128, 4), f32)
        for hj in range(2):
            for b in range(2):
                acc = np.zeros(128, f32)
                for hi in range(2):
                    acc += wexp[:, hi, hj * 128 : (hj + 1) * 128].T @ hcol[:, 2 * hi + b]
                pf[:, 2 * hj + b] = acc
        ring[:, :, BLK + k] = bf(pf * ec2[:, :, t])
        if k == PROBE_K and n < nblk - 1:
            s4 = hcol.sum(axis=0, dtype=f32)  # [4] over partitions
            sout[0, :, n] = s4[0:2] + s4[2:4]
        if k == BLK - 1 and n < nblk - 1:
            inv = (1.0 / sout[0, :, n]).astype(f32)
            # g order (h,b): b = g % 2
            frmu = bf(fr[:, :, n] * inv[None, [0, 1, 0, 1]])
            ring[:, :, 0:BLK] = bf(ring[:, :, BLK : 2 * BLK] * frmu[:, :, None])
            ring[:, :, BLK : 2 * BLK] = 0.0
    return np.log(gbuf * ec1x), sout


# ---------------------------------------------------------------- host postprocess
def _postprocess(raw_list, sp_list, t_steps=T):
    """raw: [128, 4, t] log(G*EC1*2^-40) per core; sp: [1, 2, nblk] probe sums."""
    nblk = t_steps // BLK
    shift = np.float32(40.0 * np.log(2.0))
    alphas = np.empty((B, t_steps, K), dtype=np.float32)
    for c in range(NCORES):
        raw = np.asarray(raw_list[c])  # [128, 4, t]
        sp = np.asarray(sp_list[c])  # [1, 2, nblk]
        logs = np.zeros((2, nblk), dtype=np.float64)
        logs[:, 1:] = np.log(np.maximum(sp[0, :, : nblk - 1], 1e-300))
        mu = (np.cumsum(logs, axis=1) + 40.0 * np.log(2.0)).astype(np.float32)
        nidx = np.arange(t_steps) // BLK
        for b in range(2):
            gb = SPC * c + b
            # [p, h, t] -> [t, h*128+p]
            lg = raw[:, [2 * 0 + b, 2 * 1 + b], :].transpose(2, 1, 0).reshape(t_steps, K)
            alphas[gb] = lg + mu[b, nidx][:, None]
    last = alphas[:, -1, :].astype(np.float64)
    m = last.max(axis=1)
    loglik = (np.log(np.exp(last - m[:, None]).sum(axis=1)) + m).astype(np.float32)
    return loglik, alphas


# ---------------------------------------------------------------- entry point
_CACHE = {}


def _run(log_B, pi_logits, A_logits, D_logits, trace=False, trace_kwargs=None):
    from concourse.bass_utils import run_bass_kernel_spmd

    t_steps = T
    per_core, host = _precompute(log_B, pi_logits, A_logits, D_logits, t_steps)
    if "nc" not in _CACHE:
        _CACHE["nc"] = _build(t_steps, debug=False)
    nc = _CACHE["nc"]
    in_maps = [per_core[c] for c in range(NCORES)]
    out = run_bass_kernel_spmd(
        nc, in_maps, list(range(NCORES)), trace=trace, **(trace_kwargs or {})
    )
    res = out.results
    raw_list = [res[c]["alpha_raw"] for c in range(NCORES)]
    sp_list = [res[c]["sprobe"] for c in range(NCORES)]
    loglik, alphas = _postprocess(raw_list, sp_list, t_steps)
    return loglik, alphas, out


def kernel(log_B, pi_logits, A_logits, D_logits):
    loglik, alphas, _ = _run(log_B, pi_logits, A_logits, D_logits)
    return loglik, alphas


def _ensure_ntff_hook():
    """Register the axon NTFF profile hook if the image lacks antenv.axon_hooks."""
    import types

    try:
        from antenv.axon_hooks import get_axon_ntff_profile_hook  # noqa: F401

        return
    except ImportError:
        pass
    if "/root/.axon_site" not in sys.path:
        sys.path.insert(0, "/root/.axon_site")
    from trn_agent_boot.trn_boot import _ntff_profile_via_ctypes

    import antenv

    hook = _ntff_profile_via_ctypes("/opt/axon/libaxon_pjrt.so")
    mod = types.ModuleType("antenv.axon_hooks")
    holder = {"h": hook}
    mod.get_axon_ntff_profile_hook = lambda: holder["h"]
    mod.set_axon_ntff_profile_hook = lambda h: holder.__setitem__("h", h)
    sys.modules["antenv.axon_hooks"] = mod
    antenv.axon_hooks = mod


def profile_exec_ns(log_B, pi_logits, A_logits, D_logits, tmpdir=None):
    """Run with NTFF tracing; returns HW exec time in ns (or None)."""
    _ensure_ntff_hook()
    from concourse import bass_utils as _bu

    if not getattr(_bu.upload_artifacts, "_patched", False):
        def _no_upload(tmpdir_):
            return "local://" + str(tmpdir_)

        _no_upload._patched = True
        _bu.upload_artifacts = _no_upload
    kw = {"tmpdir": tmpdir} if tmpdir else {}
    _, _, out = _run(log_B, pi_logits, A_logits, D_logits, trace=True, trace_kwargs=kw)
    return out.exec_time_ns
